# revision 17
# baseline (speedup 1.0000x reference)
"""Trainium2 Bass kernel for nn_BoxDetectionLoss (8-core data parallel).

Math: reference loss = sum_{a,r,c}[ has_match ? coord+conf_loss : conf^2 ] / denom.
A pixel (r,c) can only match a target box t if r==tb[t,0] and c==tb[t,1]
(T=16 boxes per image), so the dense term is just sum sigmoid(conf_ch)^2 over
channels {2,5,8}; the match term is a correction at <=16 pixels x 3 anchors,
computed from 144 gathered elements per image.

Each of the 8 cores handles one batch image.  v2 layout:
  - dense conf channels stream in NCHUNK-per-channel chunks, alternating
    between the sync HWDGE ring and the gpsimd SWDGE ring so both DMA rings
    pull concurrently; ACT runs sigmoid per chunk as it lands and DVE does a
    fused square+reduce (tensor_tensor_reduce) per chunk into one ACC column.
  - all tiny correction constants (tb/tp/tri/choff/TBrep, pre-converted to
    f32 on host) arrive in ONE [16,94] DMA on the sync ring ahead of the
    dense chunks; the correction chain (gather offsets, indirect gather,
    pred/match/dup math) runs on DVE in the shadow of the dense stream.
  - final: DVE reduces ACC -> [128,1], DMA to HBM; host sums 128x8 partials.
"""

import numpy as np

B, C, H, W = 8, 9, 512, 512
T = 16
N_CORES = 8
CONF_CH = (2, 5, 8)
DENOM = float(B * H * W * 3)
MAGIC = 12582912.0  # 1.5 * 2^23: x+MAGIC-MAGIC rounds to nearest-even int

import os
DENSE_MODE = os.environ.get("DENSE_MODE", "sp_gp")
SQ_MODE = os.environ.get("SQ_MODE", "tt_red")   # tt_red only (ttr hangs HW)
OUT_MODE = os.environ.get("OUT_MODE", "vec")    # vec | pe
CORR = os.environ.get("CORR", "1") == "1"
# per-half square engine: d=DVE tt+reduce, p=Pool tt+reduce, a=ACT Square+accum
SQ_ASSIGN = os.environ.get("SQ_ASSIGN", "ddddpp")

NCHUNK = 2                      # halves per channel
NDENSE = len(CONF_CH) * NCHUNK  # dense ACC columns
CCOL = NDENSE                   # correction column
CHUNK_COLS = 2048 // NCHUNK

# packed constant block: [T, 94] f32
#   0:4   tb (as float)
#   4     tp
#   5:21  tri (strictly-lower mask)
#   21:30 choff (ch*H*W)
#   30:94 TBrep (tb flattened, broadcast to all rows)
CST_COLS = 94


def make_cst(tb_i, tp_i):
    cst = np.zeros((T, CST_COLS), dtype=np.float32)
    tbf = tb_i.astype(np.float32)
    cst[:, 0:4] = tbf
    cst[:, 4] = tp_i
    cst[:, 5:21] = np.tril(np.ones((T, T), dtype=np.float32), -1)
    cst[:, 21:30] = (np.arange(C, dtype=np.float32) * (H * W))[None, :]
    cst[:, 30:94] = tbf.reshape(1, 4 * T)
    return cst


_PROG = None


def _build_correction_a(nc, sp, bass, mybir, CST, pol, gather=True):
    f32 = mybir.dt.float32
    i32 = mybir.dt.int32
    ALU = mybir.AluOpType

    TBf = CST[:, 0:4]
    TP = CST[:, 4:5]
    TRI = CST[:, 5:21]
    CH = CST[:, 21:30]
    rep4 = CST[:, 30:94].rearrange("p (t f) -> p f t", f=4)

    # packed coords: p1 = r*512 + c, p2 = r2*512 + c2 (exact in f32)
    p1 = sp.tile([T, 1], f32)
    nc.vector.tensor_scalar(
        out=p1[:], in0=TBf[:, 0:1], scalar1=512.0, scalar2=TBf[:, 1:2],
        op0=ALU.mult, op1=ALU.add,
    )
    p2 = sp.tile([T, 1], f32)
    nc.vector.tensor_scalar(
        out=p2[:], in0=TBf[:, 2:3], scalar1=512.0, scalar2=TBf[:, 3:4],
        op0=ALU.mult, op1=ALU.add,
    )

    # row-layout packed coords of all boxes, from the replicated copy
    p1row = sp.tile([T, T], f32)
    nc.vector.tensor_scalar(
        out=p1row[:], in0=rep4[:, 0, :], scalar1=512.0, scalar2=None,
        op0=ALU.mult,
    )
    nc.vector.tensor_tensor(
        out=p1row[:], in0=p1row[:], in1=rep4[:, 1, :], op=ALU.add
    )
    p2row = sp.tile([T, T], f32)
    nc.vector.tensor_scalar(
        out=p2row[:], in0=rep4[:, 2, :], scalar1=512.0, scalar2=None,
        op0=ALU.mult,
    )
    nc.vector.tensor_tensor(
        out=p2row[:], in0=p2row[:], in1=rep4[:, 3, :], op=ALU.add
    )

    # duplicate-box detection: S[t,t'] = (p1 equal) & (p2 equal), t' < t
    S = sp.tile([T, T], f32)
    nc.vector.tensor_scalar(
        out=S[:], in0=p1row[:], scalar1=p1[:], scalar2=None, op0=ALU.is_equal
    )
    S2 = sp.tile([T, T], f32)
    nc.vector.tensor_scalar(
        out=S2[:], in0=p2row[:], scalar1=p2[:], scalar2=None, op0=ALU.is_equal
    )
    nc.vector.tensor_tensor(out=S[:], in0=S[:], in1=S2[:], op=ALU.mult)
    nc.vector.tensor_tensor(out=S[:], in0=S[:], in1=TRI[:], op=ALU.mult)
    dupc = sp.tile([T, 1], f32)
    nc.vector.tensor_reduce(
        out=dupc[:], in_=S[:], axis=mybir.AxisListType.X, op=ALU.add
    )
    keep = sp.tile([T, 1], f32)
    nc.vector.tensor_scalar(
        out=keep[:], in0=dupc[:], scalar1=0.0, scalar2=None, op0=ALU.is_equal
    )

    # gather pol[ch, tb0[t], tb1[t]] for all (t, ch): offsets = ch*H*W + p1
    OFFf = sp.tile([T, C], f32)
    nc.vector.tensor_scalar(
        out=OFFf[:], in0=CH[:], scalar1=p1[:], scalar2=None, op0=ALU.add
    )
    OFFi = sp.tile([T, C], i32)
    nc.vector.tensor_copy(OFFi[:], OFFf[:])
    G = sp.tile([T, C], f32)
    if gather:
        nc.gpsimd.indirect_dma_start(
            out=G[:], out_offset=None,
            in_=pol.rearrange("c h (w a) -> (c h w) a", a=1),
            in_offset=bass.IndirectOffsetOnAxis(ap=OFFi[:], axis=0),
        )
    else:
        nc.vector.memset(G[:], 0.0)
    return dict(TBf=TBf, TP=TP, keep=keep, G=G)


def _build_correction_b(nc, sp, ACC, bass, mybir, ctx):
    f32 = mybir.dt.float32
    ALU = mybir.AluOpType
    ACT_F = mybir.ActivationFunctionType
    TP, TBf, keep, G = ctx["TP"], ctx["TBf"], ctx["keep"], ctx["G"]

    GS = sp.tile([T, C], f32)
    nc.scalar.activation(GS[:], G[:], ACT_F.Sigmoid)
    # channel ch = 3a + k: k=0 delta_r, k=1 delta_c, k=2 conf
    gs3 = GS[:].rearrange("p (a k) -> p k a", k=3)

    # pred = clip(tb + sigmoid*scale, 0, 511), all 3 anchors at once
    predr = sp.tile([T, 3], f32)
    nc.vector.tensor_scalar(
        out=predr[:], in0=gs3[:, 0, :], scalar1=9.0, scalar2=TBf[:, 0:1],
        op0=ALU.mult, op1=ALU.add,
    )
    nc.vector.tensor_scalar(
        out=predr[:], in0=predr[:], scalar1=511.0, scalar2=0.0,
        op0=ALU.min, op1=ALU.max,
    )
    predc = sp.tile([T, 3], f32)
    nc.vector.tensor_scalar(
        out=predc[:], in0=gs3[:, 1, :], scalar1=16.0, scalar2=TBf[:, 1:2],
        op0=ALU.mult, op1=ALU.add,
    )
    nc.vector.tensor_scalar(
        out=predc[:], in0=predc[:], scalar1=511.0, scalar2=0.0,
        op0=ALU.min, op1=ALU.max,
    )

    # round to nearest-even integer: (x + 1.5*2^23) - 1.5*2^23
    rr = sp.tile([T, 3], f32)
    nc.vector.tensor_scalar(
        out=rr[:], in0=predr[:], scalar1=MAGIC, scalar2=None, op0=ALU.add
    )
    nc.vector.tensor_scalar(
        out=rr[:], in0=rr[:], scalar1=MAGIC, scalar2=None, op0=ALU.subtract
    )
    rc = sp.tile([T, 3], f32)
    nc.vector.tensor_scalar(
        out=rc[:], in0=predc[:], scalar1=MAGIC, scalar2=None, op0=ALU.add
    )
    nc.vector.tensor_scalar(
        out=rc[:], in0=rc[:], scalar1=MAGIC, scalar2=None, op0=ALU.subtract
    )

    # match mask per (t, anchor)
    m = sp.tile([T, 3], f32)
    nc.vector.tensor_scalar(
        out=m[:], in0=rr[:], scalar1=TBf[:, 2:3], scalar2=None, op0=ALU.is_equal
    )
    m2 = sp.tile([T, 3], f32)
    nc.vector.tensor_scalar(
        out=m2[:], in0=rc[:], scalar1=TBf[:, 3:4], scalar2=None, op0=ALU.is_equal
    )
    nc.vector.tensor_tensor(out=m[:], in0=m[:], in1=m2[:], op=ALU.mult)

    # contribution = |predr-tb2| + |predc-tb3| + tp*(tp-2*conf)
    # |x| as max(x, -x) on DVE - keeps Abs out of the ACT function table
    d1 = sp.tile([T, 3], f32)
    nc.vector.tensor_scalar(
        out=d1[:], in0=predr[:], scalar1=TBf[:, 2:3], scalar2=None,
        op0=ALU.subtract,
    )
    d1n = sp.tile([T, 3], f32)
    nc.vector.tensor_scalar(
        out=d1n[:], in0=d1[:], scalar1=-1.0, scalar2=None, op0=ALU.mult
    )
    nc.vector.tensor_tensor(out=d1[:], in0=d1[:], in1=d1n[:], op=ALU.max)
    d2 = sp.tile([T, 3], f32)
    nc.vector.tensor_scalar(
        out=d2[:], in0=predc[:], scalar1=TBf[:, 3:4], scalar2=None,
        op0=ALU.subtract,
    )
    d2n = sp.tile([T, 3], f32)
    nc.vector.tensor_scalar(
        out=d2n[:], in0=d2[:], scalar1=-1.0, scalar2=None, op0=ALU.mult
    )
    nc.vector.tensor_tensor(out=d2[:], in0=d2[:], in1=d2n[:], op=ALU.max)
    nc.vector.tensor_tensor(out=d1[:], in0=d1[:], in1=d2[:], op=ALU.add)
    cf = sp.tile([T, 3], f32)
    nc.vector.tensor_scalar(
        out=cf[:], in0=gs3[:, 2, :], scalar1=-2.0, scalar2=TP[:],
        op0=ALU.mult, op1=ALU.add,
    )
    nc.vector.tensor_scalar(
        out=cf[:], in0=cf[:], scalar1=TP[:], scalar2=None, op0=ALU.mult
    )
    nc.vector.tensor_tensor(out=d1[:], in0=d1[:], in1=cf[:], op=ALU.add)
    # valid = match * keep; corr contribution = valid * d1
    nc.vector.tensor_scalar(
        out=m[:], in0=m[:], scalar1=keep[:], scalar2=None, op0=ALU.mult
    )
    nc.vector.tensor_tensor(out=m[:], in0=m[:], in1=d1[:], op=ALU.mult)
    nc.vector.tensor_reduce(
        out=ACC[0:T, CCOL : CCOL + 1], in_=m[:],
        axis=mybir.AxisListType.X, op=ALU.add,
    )


def _build_program(corr=CORR, gather=True, dense_mode=DENSE_MODE,
                   corr_after=2):
    import concourse.bass as bass
    import concourse.tile as tile
    from concourse import bacc, mybir

    f32 = mybir.dt.float32
    ALU = mybir.AluOpType
    ACT_F = mybir.ActivationFunctionType

    nc = bacc.Bacc(
        "TRN2", target_bir_lowering=False, debug=False, num_devices=N_CORES
    )
    pol = nc.dram_tensor("pol", [C, H, W], f32, kind="ExternalInput").ap()
    cst = nc.dram_tensor("cst", [T, CST_COLS], f32, kind="ExternalInput").ap()
    out = nc.dram_tensor(
        "out", [128] if OUT_MODE == "vec" else [1], f32, kind="ExternalOutput"
    ).ap()

    with tile.TileContext(nc) as tc:
        with (
            tc.tile_pool(name="io", bufs=1) as io,
            tc.tile_pool(name="acc", bufs=1) as accp,
            tc.tile_pool(name="small", bufs=1) as sp,
        ):
            ACC = accp.tile([128, NDENSE + 1], f32)

            # ---------- constants first on the sync ring (tiny) ----------
            CSTt = sp.tile([T, CST_COLS], f32)
            nc.sync.dma_start(CSTt[:], cst[:])

            # ---------- dense half-channel DMAs: even halves on sync HWDGE,
            # odd halves on gpsimd SWDGE (second ring); no DMA issues on the
            # ACT queue so table loads + sigmoids are never blocked ----------
            views = [
                pol[ch].rearrange("(p a) w -> p (a w)", p=128) for ch in CONF_CH
            ]
            nchunks = len(CONF_CH) * NCHUNK
            ctiles = []
            for ci in range(nchunks):
                ctiles.append(
                    io.tile([128, CHUNK_COLS], f32, name=f"in{ci}", tag=f"in{ci}")
                )

            def issue(ci, engine):
                ch, k = divmod(ci, NCHUNK)
                cols = slice(k * CHUNK_COLS, (k + 1) * CHUNK_COLS)
                engine.dma_start(ctiles[ci][:], views[ch][:, cols])

            if dense_mode == "sp_gp":
                even_eng, odd_eng = nc.sync, nc.gpsimd
            elif dense_mode == "sp_act":
                even_eng, odd_eng = nc.sync, nc.scalar
            else:  # all on sync
                even_eng, odd_eng = nc.sync, nc.sync

            for ci in range(nchunks):
                issue(ci, even_eng if ci % 2 == 0 else odd_eng)

            # memset correction column (corr only writes partitions 0..T-1)
            nc.vector.memset(ACC[:, CCOL : CCOL + 1], 0.0)

            if corr:
                corr_ctx = _build_correction_a(
                    nc, sp, bass, mybir, CSTt, pol, gather=gather
                )

            # ---------- dense compute: per half, ACT sigmoid; square+reduce
            # on the engine from SQ_ASSIGN ----------
            sigs = []
            for ci in range(nchunks):
                s = io.tile([128, CHUNK_COLS], f32, name=f"sig{ci}", tag=f"sig{ci}")
                sigs.append(s)

            n_pool = SQ_ASSIGN.count("p")
            PACC = sp.tile([1, max(n_pool, 1)], f32)
            pool_j = [0]

            def square(ci):
                eng = SQ_ASSIGN[ci]
                if eng == "a":
                    nc.scalar.activation(
                        ctiles[ci][:], sigs[ci][:], ACT_F.Square,
                        accum_out=ACC[:, ci : ci + 1],
                    )
                    return
                if eng == "p":
                    # Pool: elementwise square then full XYZWC reduce -> [1,1]
                    nc.gpsimd.tensor_tensor(
                        out=ctiles[ci][:], in0=sigs[ci][:], in1=sigs[ci][:],
                        op=ALU.mult,
                    )
                    j = pool_j[0]
                    nc.gpsimd.tensor_reduce(
                        out=PACC[0:1, j : j + 1], in_=ctiles[ci][:],
                        axis=mybir.AxisListType.XYZWC, op=ALU.add,
                    )
                    pool_j[0] += 1
                    nc.vector.memset(ACC[:, ci : ci + 1], 0.0)
                    return
                nc.vector.tensor_tensor(
                    out=ctiles[ci][:], in0=sigs[ci][:], in1=sigs[ci][:],
                    op=ALU.mult,
                )
                nc.vector.tensor_reduce(
                    out=ACC[:, ci : ci + 1], in_=ctiles[ci][:],
                    axis=mybir.AxisListType.X, op=ALU.add,
                )

            corr_done = not corr
            for ci in range(nchunks):
                nc.scalar.activation(sigs[ci][:], ctiles[ci][:], ACT_F.Sigmoid)
                square(ci)
                if ci + 1 == corr_after and not corr_done:
                    _build_correction_b(nc, sp, ACC, bass, mybir, corr_ctx)
                    corr_done = True
            if not corr_done:
                _build_correction_b(nc, sp, ACC, bass, mybir, corr_ctx)

            RED = sp.tile([128, 1], f32)
            nc.vector.tensor_reduce(
                out=RED[:], in_=ACC[:], axis=mybir.AxisListType.X, op=ALU.add
            )
            if n_pool:
                PR = sp.tile([1, 1], f32)
                nc.vector.tensor_reduce(
                    out=PR[:], in_=PACC[0:1, 0:n_pool],
                    axis=mybir.AxisListType.X, op=ALU.add,
                )
                nc.vector.tensor_tensor(
                    out=RED[0:1, 0:1], in0=RED[0:1, 0:1], in1=PR[:], op=ALU.add
                )
            if OUT_MODE == "vec":
                nc.sync.dma_start(out[:], RED[:])
            else:
                ONES = sp.tile([128, 1], f32)
                nc.vector.memset(ONES[:], 1.0)
                with tc.tile_pool(name="psum", bufs=1, space="PSUM") as psum:
                    PS = psum.tile([1, 1], f32, space="PSUM")
                    nc.tensor.matmul(out=PS[:], lhsT=RED[:], rhs=ONES[:],
                                     start=True, stop=True)
                    OUTSB = sp.tile([1, 1], f32)
                    nc.vector.tensor_copy(OUTSB[:], PS[:])
                    nc.scalar.dma_start(out[:], OUTSB[:])

    nc.compile()
    return nc


def get_program():
    global _PROG
    if _PROG is None:
        _PROG = _build_program()
    return _PROG


def make_in_maps(policy_output, target_boxes, target_probs):
    policy_output = np.ascontiguousarray(np.asarray(policy_output, dtype=np.float32))
    target_boxes = np.ascontiguousarray(np.asarray(target_boxes, dtype=np.int32))
    target_probs = np.ascontiguousarray(np.asarray(target_probs, dtype=np.float32))
    assert policy_output.shape == (B, C, H, W)
    in_maps = []
    for i in range(N_CORES):
        in_maps.append(
            {
                "pol": policy_output[i],
                "cst": make_cst(target_boxes[i], target_probs[i]),
            }
        )
    return in_maps


def kernel(policy_output, target_boxes, target_probs):
    from concourse.bass_utils import run_bass_kernel_spmd

    nc = get_program()
    in_maps = make_in_maps(policy_output, target_boxes, target_probs)
    res = run_bass_kernel_spmd(nc, in_maps, list(range(N_CORES)))
    total = 0.0
    for i in range(N_CORES):
        total += float(res.results[i]["out"].sum(dtype=np.float64))
    return np.float32(total / DENOM)


# revision 20
# speedup vs baseline: 1.1109x; 1.1109x over previous
"""Trainium2 Bass kernel for nn_BoxDetectionLoss (8-core data parallel).

Math: reference loss = sum_{a,r,c}[ has_match ? coord+conf_loss : conf^2 ] / denom.
A pixel (r,c) can only match a target box t if r==tb[t,0] and c==tb[t,1]
(T=16 boxes per image), so the dense term is sum sigmoid(conf_ch)^2 over
channels {2,5,8}; the match term is a correction at <=16 pixels x 3 anchors
from 144 gathered elements per image.

Each of the 8 cores handles one batch image.  v4 layout:
  - the 3 conf channels stream as 8 units (4 quarters + 4 halves) spread
    over 3 DMA rings (sync HWDGE / scalar HWDGE / gpsimd SWDGE) so the
    aggregate hits the ~358 GB/s per-core HBM limit; first/last transfers
    are quarters to cut pipeline fill/drain latency.
  - per unit: ACT sigmoid (f32), DVE square (tensor_tensor -> bf16), and
    the otherwise-idle PE reduces each squared tile with a ones-vector
    matmul, accumulating everything in one PSUM [1,512] bank.  A couple of
    units' squares can go to Pool (SQ_ASSIGN knob).
  - correction inputs that depend only on (tb, tp) - gather offsets and the
    first-duplicate keep mask - are precomputed on host, leaving a ~16-op
    DVE chain that runs in the shadow of the stream.
  - final: DVE reduces PSUM + ACC, one [128] DMA out; host sums partials.
"""

import os

import numpy as np

B, C, H, W = 8, 9, 512, 512
T = 16
N_CORES = 8
CONF_CH = (2, 5, 8)
DENOM = float(B * H * W * 3)
MAGIC = 12582912.0  # 1.5 * 2^23: x+MAGIC-MAGIC rounds to nearest-even int

# stream units: (channel-idx, col_start, col_end) over the [128, 2048] view
UNITS = (
    (0, 0, 512), (0, 512, 1024), (0, 1024, 2048),
    (1, 0, 1024), (1, 1024, 2048),
    (2, 0, 1024), (2, 1024, 1536), (2, 1536, 2048),
)
# per-unit DMA ring: s=sync HWDGE, a=scalar HWDGE, g=gpsimd SWDGE
RING_ASSIGN = os.environ.get("RING_ASSIGN", "ssgaagss")
# per-unit square engine: d=DVE tt(->bf16)+PE reduce, p=Pool tt+PE reduce,
# a=ACT Square+accum
SQ_ASSIGN = os.environ.get("SQ_ASSIGN", "dddppddd")
CORR = os.environ.get("CORR", "1") == "1"

# packed f32 constants: [T, 8] = tbf(4) | tp | keep | pad(2)
CST_COLS = 8


def make_cst(tb_i, tp_i):
    cst = np.zeros((T, CST_COLS), dtype=np.float32)
    cst[:, 0:4] = tb_i.astype(np.float32)
    cst[:, 4] = tp_i
    # keep[t] = 1 unless an earlier box t' has identical (r,c,r2,c2)
    for t in range(T):
        dup = False
        for t2 in range(t):
            if (tb_i[t] == tb_i[t2]).all():
                dup = True
                break
        cst[t, 5] = 0.0 if dup else 1.0
    return cst


def make_offs(tb_i):
    # gather offsets into pol.flat: ch*H*W + r*W + c  for all (t, ch)
    base = (tb_i[:, 0].astype(np.int64) * W + tb_i[:, 1]).astype(np.int32)
    offs = base[:, None] + (np.arange(C, dtype=np.int32) * (H * W))[None, :]
    return np.ascontiguousarray(offs.astype(np.int32))


_PROG = None


def _build_correction(nc, sp, ACC, ccol, bass, mybir, CST, G):
    f32 = mybir.dt.float32
    ALU = mybir.AluOpType
    ACT_F = mybir.ActivationFunctionType

    TBf = CST[:, 0:4]
    TP = CST[:, 4:5]
    KEEP = CST[:, 5:6]

    GS = sp.tile([T, C], f32)
    nc.scalar.activation(GS[:], G[:], ACT_F.Sigmoid)
    # channel ch = 3a + k: k=0 delta_r, k=1 delta_c, k=2 conf
    gs3 = GS[:].rearrange("p (a k) -> p k a", k=3)

    # pred = clip(tb + sigmoid*scale, 0, 511)
    predr = sp.tile([T, 3], f32)
    nc.vector.tensor_scalar(
        out=predr[:], in0=gs3[:, 0, :], scalar1=9.0, scalar2=TBf[:, 0:1],
        op0=ALU.mult, op1=ALU.add,
    )
    nc.vector.tensor_scalar(
        out=predr[:], in0=predr[:], scalar1=511.0, scalar2=0.0,
        op0=ALU.min, op1=ALU.max,
    )
    predc = sp.tile([T, 3], f32)
    nc.vector.tensor_scalar(
        out=predc[:], in0=gs3[:, 1, :], scalar1=16.0, scalar2=TBf[:, 1:2],
        op0=ALU.mult, op1=ALU.add,
    )
    nc.vector.tensor_scalar(
        out=predc[:], in0=predc[:], scalar1=511.0, scalar2=0.0,
        op0=ALU.min, op1=ALU.max,
    )

    # round-half-even in one op: (x + 1.5*2^23) - 1.5*2^23
    rr = sp.tile([T, 3], f32)
    nc.vector.tensor_scalar(
        out=rr[:], in0=predr[:], scalar1=MAGIC, scalar2=MAGIC,
        op0=ALU.add, op1=ALU.subtract,
    )
    rc = sp.tile([T, 3], f32)
    nc.vector.tensor_scalar(
        out=rc[:], in0=predc[:], scalar1=MAGIC, scalar2=MAGIC,
        op0=ALU.add, op1=ALU.subtract,
    )

    # match mask: (rr==tb2) * (rc==tb3)
    m2 = sp.tile([T, 3], f32)
    nc.vector.tensor_scalar(
        out=m2[:], in0=rc[:], scalar1=TBf[:, 3:4], scalar2=None,
        op0=ALU.is_equal,
    )
    m = sp.tile([T, 3], f32)
    nc.vector.scalar_tensor_tensor(
        out=m[:], in0=rr[:], scalar=TBf[:, 2:3], in1=m2[:],
        op0=ALU.is_equal, op1=ALU.mult,
    )

    # coord = |predr-tb2| + |predc-tb3|; |x| as max(x, -x)
    d1 = sp.tile([T, 3], f32)
    nc.vector.tensor_scalar(
        out=d1[:], in0=predr[:], scalar1=TBf[:, 2:3], scalar2=None,
        op0=ALU.subtract,
    )
    d1n = sp.tile([T, 3], f32)
    nc.vector.tensor_scalar(
        out=d1n[:], in0=d1[:], scalar1=-1.0, scalar2=None, op0=ALU.mult
    )
    nc.vector.tensor_tensor(out=d1[:], in0=d1[:], in1=d1n[:], op=ALU.max)
    d2 = sp.tile([T, 3], f32)
    nc.vector.tensor_scalar(
        out=d2[:], in0=predc[:], scalar1=TBf[:, 3:4], scalar2=None,
        op0=ALU.subtract,
    )
    d2n = sp.tile([T, 3], f32)
    nc.vector.tensor_scalar(
        out=d2n[:], in0=d2[:], scalar1=-1.0, scalar2=None, op0=ALU.mult
    )
    nc.vector.tensor_tensor(out=d2[:], in0=d2[:], in1=d2n[:], op=ALU.max)
    # conf part: tp*(tp - 2*conf); total = d1 + d2 + cf
    cf = sp.tile([T, 3], f32)
    nc.vector.tensor_scalar(
        out=cf[:], in0=gs3[:, 2, :], scalar1=-2.0, scalar2=TP[:],
        op0=ALU.mult, op1=ALU.add,
    )
    nc.vector.scalar_tensor_tensor(
        out=cf[:], in0=cf[:], scalar=TP[:], in1=d2[:],
        op0=ALU.mult, op1=ALU.add,
    )
    nc.vector.tensor_tensor(out=d1[:], in0=d1[:], in1=cf[:], op=ALU.add)
    # contribution = m * keep * total
    nc.vector.scalar_tensor_tensor(
        out=m[:], in0=m[:], scalar=KEEP[:], in1=d1[:],
        op0=ALU.mult, op1=ALU.mult,
    )
    nc.vector.tensor_reduce(
        out=ACC[0:T, ccol : ccol + 1], in_=m[:],
        axis=mybir.AxisListType.X, op=ALU.add,
    )


def _build_program(corr=CORR, ring_assign=RING_ASSIGN, sq_assign=SQ_ASSIGN):
    import concourse.bass as bass
    import concourse.tile as tile
    from concourse import bacc, mybir

    f32 = mybir.dt.float32
    bf16 = mybir.dt.bfloat16
    i32 = mybir.dt.int32
    ALU = mybir.AluOpType
    ACT_F = mybir.ActivationFunctionType

    nc = bacc.Bacc(
        "TRN2", target_bir_lowering=False, debug=False, num_devices=N_CORES
    )
    pol = nc.dram_tensor("pol", [C, H, W], f32, kind="ExternalInput").ap()
    cst = nc.dram_tensor("cst", [T, CST_COLS], f32, kind="ExternalInput").ap()
    offs = nc.dram_tensor("offs", [T, C], i32, kind="ExternalInput").ap()
    out = nc.dram_tensor("out", [128], f32, kind="ExternalOutput").ap()

    ring_eng = {"s": "sync", "a": "scalar", "g": "gpsimd"}
    n_units = len(UNITS)
    n_a = sq_assign.count("a")

    with tile.TileContext(nc) as tc:
        with (
            tc.tile_pool(name="io", bufs=1) as io,
            tc.tile_pool(name="acc", bufs=1) as accp,
            tc.tile_pool(name="small", bufs=1) as sp,
            tc.tile_pool(name="psum", bufs=1, space="PSUM") as psum,
        ):
            ACC = accp.tile([128, n_a + 1], f32)
            ccol = n_a

            # ---------- tiny constant DMAs first on the sync ring ----------
            CSTt = sp.tile([T, CST_COLS], f32)
            nc.sync.dma_start(CSTt[:], cst[:])
            OFFt = sp.tile([T, C], i32)
            nc.sync.dma_start(OFFt[:], offs[:])

            # ---------- dense unit DMAs across 3 rings ----------
            views = [
                pol[ch].rearrange("(p a) w -> p (a w)", p=128) for ch in CONF_CH
            ]
            utiles = []
            for ui, (ch, c0, c1) in enumerate(UNITS):
                utiles.append(
                    io.tile([128, c1 - c0], f32, name=f"in{ui}", tag=f"in{ui}")
                )
            for ui, (ch, c0, c1) in enumerate(UNITS):
                eng = getattr(nc, ring_eng[ring_assign[ui]])
                eng.dma_start(utiles[ui][:], views[ch][:, c0:c1])

            # memset correction column (corr only writes partitions 0..T-1)
            nc.vector.memset(ACC[:, ccol : ccol + 1], 0.0)

            # indirect gather of the 144 candidate elements (SWDGE)
            G = sp.tile([T, C], f32)
            if corr:
                nc.gpsimd.indirect_dma_start(
                    out=G[:], out_offset=None,
                    in_=pol.rearrange("c h (w a) -> (c h w) a", a=1),
                    in_offset=bass.IndirectOffsetOnAxis(ap=OFFt[:], axis=0),
                )

            # ---------- dense compute ----------
            sigs = []
            for ui, (ch, c0, c1) in enumerate(UNITS):
                s = io.tile([128, c1 - c0], f32, name=f"sg{ui}", tag=f"sg{ui}")
                sigs.append(s)
            sqs = []
            for ui, (ch, c0, c1) in enumerate(UNITS):
                s = io.tile([128, c1 - c0], bf16, name=f"sq{ui}", tag=f"sq{ui}")
                sqs.append(s)

            PACC = psum.tile([1, 512], f32, space="PSUM")
            ONESB = sp.tile([128, 1], bf16)
            nc.vector.memset(ONESB[:], 1.0)

            # ACT queue: sigmoids in unit order; gather-sigmoid after unit 0
            a_col = [0]

            def act_square(ui):
                nc.scalar.activation(
                    utiles[ui][:], sigs[ui][:], ACT_F.Square,
                    accum_out=ACC[:, a_col[0] : a_col[0] + 1],
                )
                a_col[0] += 1

            def dve_or_pool_square(ui, e):
                e.tensor_tensor(
                    out=sqs[ui][:], in0=sigs[ui][:], in1=sigs[ui][:],
                    op=ALU.mult,
                )

            # issue all sigmoids + squares in unit order; matmuls trail on PE
            mm_units = [ui for ui in range(n_units) if sq_assign[ui] != "a"]
            mm_total = sum(
                (UNITS[ui][2] - UNITS[ui][1] + 511) // 512 for ui in mm_units
            )
            mm_done = [0]

            def pe_reduce(ui):
                ch, c0, c1 = UNITS[ui]
                w = c1 - c0
                for j in range(0, w, 512):
                    first = mm_done[0] == 0
                    mm_done[0] += 1
                    last = mm_done[0] == mm_total
                    nc.tensor.matmul(
                        out=PACC[:], lhsT=ONESB[:],
                        rhs=sqs[ui][:, j : j + 512],
                        start=first, stop=last,
                    )

            for ui in range(n_units):
                nc.scalar.activation(sigs[ui][:], utiles[ui][:], ACT_F.Sigmoid)
                if ui == 0 and corr:
                    _build_correction(nc, sp, ACC, ccol, bass, mybir, CSTt, G)
                kind = sq_assign[ui]
                if kind == "a":
                    act_square(ui)
                elif kind == "p":
                    dve_or_pool_square(ui, nc.gpsimd)
                    pe_reduce(ui)
                else:
                    dve_or_pool_square(ui, nc.vector)
                    pe_reduce(ui)

            # ---------- final merge ----------
            RED = sp.tile([128, 1], f32)
            nc.vector.tensor_reduce(
                out=RED[:], in_=ACC[:], axis=mybir.AxisListType.X, op=ALU.add
            )
            PR = sp.tile([1, 1], f32)
            nc.vector.tensor_reduce(
                out=PR[:], in_=PACC[:], axis=mybir.AxisListType.X, op=ALU.add
            )
            nc.vector.tensor_tensor(
                out=RED[0:1, 0:1], in0=RED[0:1, 0:1], in1=PR[:], op=ALU.add
            )
            nc.sync.dma_start(out[:], RED[:])

    nc.compile()
    return nc


def get_program():
    global _PROG
    if _PROG is None:
        _PROG = _build_program()
    return _PROG


def make_in_maps(policy_output, target_boxes, target_probs):
    policy_output = np.ascontiguousarray(np.asarray(policy_output, dtype=np.float32))
    target_boxes = np.ascontiguousarray(np.asarray(target_boxes, dtype=np.int32))
    target_probs = np.ascontiguousarray(np.asarray(target_probs, dtype=np.float32))
    assert policy_output.shape == (B, C, H, W)
    in_maps = []
    for i in range(N_CORES):
        in_maps.append(
            {
                "pol": policy_output[i],
                "cst": make_cst(target_boxes[i], target_probs[i]),
                "offs": make_offs(target_boxes[i]),
            }
        )
    return in_maps


def kernel(policy_output, target_boxes, target_probs):
    from concourse.bass_utils import run_bass_kernel_spmd

    nc = get_program()
    in_maps = make_in_maps(policy_output, target_boxes, target_probs)
    res = run_bass_kernel_spmd(nc, in_maps, list(range(N_CORES)))
    total = 0.0
    for i in range(N_CORES):
        total += float(res.results[i]["out"].sum(dtype=np.float64))
    return np.float32(total / DENOM)


# revision 21
# speedup vs baseline: 1.3977x; 1.2581x over previous
"""Trainium2 Bass kernel for nn_BoxDetectionLoss (8-core data parallel).

Math: reference loss = sum_{a,r,c}[ has_match ? coord+conf_loss : conf^2 ] / denom.
A pixel (r,c) can only match a target box t if r==tb[t,0] and c==tb[t,1]
(T=16 boxes per image), so the dense term is sum sigmoid(conf_ch)^2 over
channels {2,5,8}; the match term is a correction at <=16 pixels x 3 anchors
from 144 gathered elements per image.

Each of the 8 cores handles one batch image.  v5 layout:
  - each conf channel streams as 2 partition-split halves ([0:64],[64:128],
    keeping the fast 8KB-contiguous-per-partition DMA descriptor shape) on
    the two HWDGE rings, laddered ch0 -> ch1 -> ch2 so compute pipelines.
  - ACT runs sigmoid per channel (f32 -> bf16 out); ch0/ch1 squares are DVE
    tensor_tensor in bf16 (2x column rate) and the idle PE reduces them via
    ones-vector matmuls accumulated in one PSUM [1,512] bank; ch2 (the
    tail) splits square work between ACT Square+accum and DVE tt+reduce.
  - all correction inputs that depend only on (tb, tp) - gather offsets and
    the duplicate-box keep mask - are precomputed on the host and packed
    with tb/tp into ONE tiny f32 DMA; the ~16-op DVE correction chain and
    the 144-element SWDGE gather run in the shadow of the dense stream.
  - final: PSUM + ACC partials merge into [128] f32, one DMA out; host sums.
"""

import os

import numpy as np

B, C, H, W = 8, 9, 512, 512
T = 16
N_CORES = 8
CONF_CH = (2, 5, 8)
DENOM = float(B * H * W * 3)
MAGIC = 12582912.0  # 1.5 * 2^23: x+MAGIC-MAGIC rounds to nearest-even int

SPL = int(os.environ.get("SPL", "1280"))  # ch2 square split: ACT [0:SPL], DVE rest
CORR = os.environ.get("CORR", "1") == "1"

# packed f32 constants: [T, 16] = tbf(4) | tp | keep | offs(9, exact ints) | pad
CST_COLS = 16


def make_cst(tb_i, tp_i):
    cst = np.zeros((T, CST_COLS), dtype=np.float32)
    cst[:, 0:4] = tb_i.astype(np.float32)
    cst[:, 4] = tp_i
    for t in range(T):
        dup = any((tb_i[t] == tb_i[t2]).all() for t2 in range(t))
        cst[t, 5] = 0.0 if dup else 1.0
    base = tb_i[:, 0].astype(np.int64) * W + tb_i[:, 1]
    offs = base[:, None] + np.arange(C, dtype=np.int64)[None, :] * (H * W)
    cst[:, 6:15] = offs.astype(np.float32)  # < 2^24, exact in f32
    return cst


_PROG = None


def _build_correction(nc, sp, ACC, ccol, bass, mybir, CST, G):
    f32 = mybir.dt.float32
    ALU = mybir.AluOpType
    ACT_F = mybir.ActivationFunctionType

    TBf = CST[:, 0:4]
    TP = CST[:, 4:5]
    KEEP = CST[:, 5:6]

    GS = sp.tile([T, C], f32)
    nc.scalar.activation(GS[:], G[:], ACT_F.Sigmoid)
    # channel ch = 3a + k: k=0 delta_r, k=1 delta_c, k=2 conf
    gs3 = GS[:].rearrange("p (a k) -> p k a", k=3)

    # pred = clip(tb + sigmoid*scale, 0, 511)
    predr = sp.tile([T, 3], f32)
    nc.vector.tensor_scalar(
        out=predr[:], in0=gs3[:, 0, :], scalar1=9.0, scalar2=TBf[:, 0:1],
        op0=ALU.mult, op1=ALU.add,
    )
    nc.vector.tensor_scalar(
        out=predr[:], in0=predr[:], scalar1=511.0, scalar2=0.0,
        op0=ALU.min, op1=ALU.max,
    )
    predc = sp.tile([T, 3], f32)
    nc.vector.tensor_scalar(
        out=predc[:], in0=gs3[:, 1, :], scalar1=16.0, scalar2=TBf[:, 1:2],
        op0=ALU.mult, op1=ALU.add,
    )
    nc.vector.tensor_scalar(
        out=predc[:], in0=predc[:], scalar1=511.0, scalar2=0.0,
        op0=ALU.min, op1=ALU.max,
    )

    # round-half-even: (x + 1.5*2^23) - 1.5*2^23
    rr = sp.tile([T, 3], f32)
    nc.vector.tensor_scalar(
        out=rr[:], in0=predr[:], scalar1=MAGIC, scalar2=None, op0=ALU.add
    )
    nc.vector.tensor_scalar(
        out=rr[:], in0=rr[:], scalar1=MAGIC, scalar2=None, op0=ALU.subtract
    )
    rc = sp.tile([T, 3], f32)
    nc.vector.tensor_scalar(
        out=rc[:], in0=predc[:], scalar1=MAGIC, scalar2=None, op0=ALU.add
    )
    nc.vector.tensor_scalar(
        out=rc[:], in0=rc[:], scalar1=MAGIC, scalar2=None, op0=ALU.subtract
    )

    # match mask: (rr==tb2) * (rc==tb3)
    m2 = sp.tile([T, 3], f32)
    nc.vector.tensor_scalar(
        out=m2[:], in0=rc[:], scalar1=TBf[:, 3:4], scalar2=None,
        op0=ALU.is_equal,
    )
    m = sp.tile([T, 3], f32)
    nc.vector.scalar_tensor_tensor(
        out=m[:], in0=rr[:], scalar=TBf[:, 2:3], in1=m2[:],
        op0=ALU.is_equal, op1=ALU.mult,
    )

    # coord = |predr-tb2| + |predc-tb3|; |x| as max(x, -x)
    d1 = sp.tile([T, 3], f32)
    nc.vector.tensor_scalar(
        out=d1[:], in0=predr[:], scalar1=TBf[:, 2:3], scalar2=None,
        op0=ALU.subtract,
    )
    d1n = sp.tile([T, 3], f32)
    nc.vector.tensor_scalar(
        out=d1n[:], in0=d1[:], scalar1=-1.0, scalar2=None, op0=ALU.mult
    )
    nc.vector.tensor_tensor(out=d1[:], in0=d1[:], in1=d1n[:], op=ALU.max)
    d2 = sp.tile([T, 3], f32)
    nc.vector.tensor_scalar(
        out=d2[:], in0=predc[:], scalar1=TBf[:, 3:4], scalar2=None,
        op0=ALU.subtract,
    )
    d2n = sp.tile([T, 3], f32)
    nc.vector.tensor_scalar(
        out=d2n[:], in0=d2[:], scalar1=-1.0, scalar2=None, op0=ALU.mult
    )
    nc.vector.tensor_tensor(out=d2[:], in0=d2[:], in1=d2n[:], op=ALU.max)
    # conf part tp*(tp-2*conf); total = d1 + (cf + d2)
    cf = sp.tile([T, 3], f32)
    nc.vector.tensor_scalar(
        out=cf[:], in0=gs3[:, 2, :], scalar1=-2.0, scalar2=TP[:],
        op0=ALU.mult, op1=ALU.add,
    )
    nc.vector.scalar_tensor_tensor(
        out=cf[:], in0=cf[:], scalar=TP[:], in1=d2[:],
        op0=ALU.mult, op1=ALU.add,
    )
    nc.vector.tensor_tensor(out=d1[:], in0=d1[:], in1=cf[:], op=ALU.add)
    # contribution = m * keep * total
    nc.vector.scalar_tensor_tensor(
        out=m[:], in0=m[:], scalar=KEEP[:], in1=d1[:],
        op0=ALU.mult, op1=ALU.mult,
    )
    nc.vector.tensor_reduce(
        out=ACC[0:T, ccol : ccol + 1], in_=m[:],
        axis=mybir.AxisListType.X, op=ALU.add,
    )


def _build_program(corr=CORR, spl=SPL):
    import concourse.bass as bass
    import concourse.tile as tile
    from concourse import bacc, mybir

    f32 = mybir.dt.float32
    bf16 = mybir.dt.bfloat16
    i32 = mybir.dt.int32
    ALU = mybir.AluOpType
    ACT_F = mybir.ActivationFunctionType

    nc = bacc.Bacc(
        "TRN2", target_bir_lowering=False, debug=False, num_devices=N_CORES
    )
    pol = nc.dram_tensor("pol", [C, H, W], f32, kind="ExternalInput").ap()
    cst = nc.dram_tensor("cst", [T, CST_COLS], f32, kind="ExternalInput").ap()
    out = nc.dram_tensor("out", [128], f32, kind="ExternalOutput").ap()

    with tile.TileContext(nc) as tc:
        with (
            tc.tile_pool(name="io", bufs=1) as io,
            tc.tile_pool(name="acc", bufs=1) as accp,
            tc.tile_pool(name="small", bufs=1) as sp,
            tc.tile_pool(name="psum", bufs=1, space="PSUM") as psum,
        ):
            # ACC cols: 0 = ch2 ACT-square accum, 1 = ch2 DVE tail, 2 = corr
            ACC = accp.tile([128, 3], f32)
            ccol = 2

            # ---------- constants: one tiny DMA on the sync ring ----------
            CSTt = sp.tile([T, CST_COLS], f32)
            nc.sync.dma_start(CSTt[:], cst[:])

            # ---------- dense: 3 channels x 2 partition-halves ----------
            views = [
                pol[ch].rearrange("(p a) w -> p (a w)", p=128) for ch in CONF_CH
            ]
            tins = []
            for k in range(3):
                tins.append(io.tile([128, 2048], f32, name=f"in{k}", tag=f"in{k}"))
            for k in range(3):
                nc.sync.dma_start(tins[k][0:64, :], views[k][0:64, :])
                nc.scalar.dma_start(tins[k][64:128, :], views[k][64:128, :])

            nc.vector.memset(ACC[:, ccol : ccol + 1], 0.0)
            nc.vector.memset(ACC[0:1, 0:2], 0.0)  # harmless; keeps sim happy

            # gather offsets -> i32, then SWDGE indirect gather
            G = sp.tile([T, C], f32)
            if corr:
                OFFi = sp.tile([T, C], i32)
                nc.vector.tensor_copy(OFFi[:], CSTt[:, 6:15])
                nc.gpsimd.indirect_dma_start(
                    out=G[:], out_offset=None,
                    in_=pol.rearrange("c h (w a) -> (c h w) a", a=1),
                    in_offset=bass.IndirectOffsetOnAxis(ap=OFFi[:], axis=0),
                )

            sigs = []
            for k in range(3):
                sigs.append(
                    io.tile([128, 2048], bf16, name=f"sg{k}", tag=f"sg{k}")
                )
            sqs = []
            for k in range(2):
                sqs.append(
                    io.tile([128, 2048], bf16, name=f"sq{k}", tag=f"sq{k}")
                )

            PACC = psum.tile([1, 512], f32, space="PSUM")
            ONESB = sp.tile([128, 1], bf16)
            nc.vector.memset(ONESB[:], 1.0)

            # ch0 / ch1: sigmoid -> DVE bf16 square -> PE ones-matmul reduce
            n_mm = 0
            for k in range(2):
                nc.scalar.activation(sigs[k][:], tins[k][:], ACT_F.Sigmoid)
                if k == 0 and corr:
                    _build_correction(nc, sp, ACC, ccol, bass, mybir, CSTt, G)
                nc.vector.tensor_tensor(
                    out=sqs[k][:], in0=sigs[k][:], in1=sigs[k][:], op=ALU.mult
                )
                for j in range(0, 2048, 512):
                    nc.tensor.matmul(
                        out=PACC[:], lhsT=ONESB[:], rhs=sqs[k][:, j : j + 512],
                        start=(n_mm == 0), stop=(n_mm == 7),
                    )
                    n_mm += 1

            # ch2 (tail): split square between ACT (accum) and DVE (tt+reduce)
            nc.scalar.activation(sigs[2][:], tins[2][:], ACT_F.Sigmoid)
            nc.scalar.activation(
                tins[2][:, 0:spl], sigs[2][:, 0:spl], ACT_F.Square,
                accum_out=ACC[:, 0:1],
            )
            SQT = sp.tile([128, 2048 - spl], bf16)
            nc.vector.tensor_tensor(
                out=SQT[:], in0=sigs[2][:, spl:], in1=sigs[2][:, spl:],
                op=ALU.mult,
            )
            nc.vector.tensor_reduce(
                out=ACC[:, 1:2], in_=SQT[:], axis=mybir.AxisListType.X,
                op=ALU.add,
            )

            # ---------- final merge ----------
            PR = sp.tile([1, 1], f32)
            nc.vector.tensor_reduce(
                out=PR[:], in_=PACC[:], axis=mybir.AxisListType.X, op=ALU.add
            )
            RED = sp.tile([128, 1], f32)
            nc.vector.tensor_reduce(
                out=RED[:], in_=ACC[:], axis=mybir.AxisListType.X, op=ALU.add
            )
            nc.vector.tensor_tensor(
                out=RED[0:1, 0:1], in0=RED[0:1, 0:1], in1=PR[:], op=ALU.add
            )
            nc.sync.dma_start(out[:], RED[:])

    nc.compile()
    return nc


def get_program():
    global _PROG
    if _PROG is None:
        _PROG = _build_program()
    return _PROG


def make_in_maps(policy_output, target_boxes, target_probs):
    policy_output = np.ascontiguousarray(np.asarray(policy_output, dtype=np.float32))
    target_boxes = np.ascontiguousarray(np.asarray(target_boxes, dtype=np.int32))
    target_probs = np.ascontiguousarray(np.asarray(target_probs, dtype=np.float32))
    assert policy_output.shape == (B, C, H, W)
    in_maps = []
    for i in range(N_CORES):
        in_maps.append(
            {
                "pol": policy_output[i],
                "cst": make_cst(target_boxes[i], target_probs[i]),
            }
        )
    return in_maps


def kernel(policy_output, target_boxes, target_probs):
    from concourse.bass_utils import run_bass_kernel_spmd

    nc = get_program()
    in_maps = make_in_maps(policy_output, target_boxes, target_probs)
    res = run_bass_kernel_spmd(nc, in_maps, list(range(N_CORES)))
    total = 0.0
    for i in range(N_CORES):
        total += float(res.results[i]["out"].sum(dtype=np.float64))
    return np.float32(total / DENOM)
